# revision 42
# baseline (speedup 1.0000x reference)
"""AnchorDeformAtt (deformable attention) on 8 TRN2 NeuronCores.

Sharding: core m -> batch b = m//4, head pair (2*(m%4), 2*(m%4)+1).
Per core:
  - fused 1x1-conv projections (value/size/anchor/att) as PE matmuls
  - per-point bilinear taps resolved via a per-head "4-tap table" in DRAM
    (row r = [P[r], P[r+1], P[r+W], P[r+W+1]], bf16, 256B rows) gathered
    with gpsimd dma_gather (one 256B row per sample point)
  - tap/point reduction: DVE multiplies + adds, then the 16-point sum is
    done on the TensorEngine as 16 accumulating matmuls against identity
    (which also transposes head_out into [head_dim, l] layout)
  - AllToAll over each 4-core batch group swaps head-shards for l-shards,
    then each core computes out_proj + BN for its quarter of L.
Host assembles the 8 (C, L/4) quarters into the full output.
"""
import sys

sys.path.insert(0, '/opt/trn_rl_repo')

import numpy as np

B, C, H, W = 2, 256, 64, 96
L = H * W              # 6144
NH, NP, HD = 8, 16, 32
NT = L // 128          # 48 l-tiles
LQ = L // 8            # 768, per-core output columns (per batch)
NPROJ = 164            # fused projection output columns
PCOLS = 100            # staged non-value projection columns
EPS = 1e-6
TWO23 = 8388608.0

_CACHE = {}


def _build_nc():
    import concourse.mybir as mybir
    import concourse.tile as tile
    from concourse import bacc
    from concourse.masks import make_identity

    fp32 = mybir.dt.float32
    bf16 = mybir.dt.bfloat16
    i16 = mybir.dt.int16
    A = mybir.AluOpType
    AF = mybir.ActivationFunctionType

    nc = bacc.Bacc("TRN2", target_bir_lowering=False, num_devices=8,
                   num_swdge_queues=4)

    feat = nc.dram_tensor("feat", [C, L], fp32, kind="ExternalInput")
    wproj = nc.dram_tensor("wproj", [C, NPROJ], fp32, kind="ExternalInput")
    bproj = nc.dram_tensor("bproj", [1, NPROJ], fp32, kind="ExternalInput")
    wot = nc.dram_tensor("wot", [C, C], bf16, kind="ExternalInput")
    bnsc = nc.dram_tensor("bnsc", [128, 2], fp32, kind="ExternalInput")
    bnbi = nc.dram_tensor("bnbi", [128, 2], fp32, kind="ExternalInput")
    cent = nc.dram_tensor("cent", [128, 96], fp32, kind="ExternalInput")
    fold = nc.dram_tensor("fold", [128, 24], fp32, kind="ExternalInput")
    out = nc.dram_tensor("out", [2 * C, LQ], fp32, kind="ExternalOutput")

    with tile.TileContext(nc) as tc:
        with (
            tc.tile_pool(name="const", bufs=1) as cpool,
            tc.tile_pool(name="pers", bufs=1) as ppool,
            tc.tile_pool(name="work", bufs=3) as wpool,
            tc.tile_pool(name="tmp", bufs=1) as tpool,
            tc.tile_pool(name="psA", bufs=3, space="PSUM") as pspool,
            tc.tile_pool(name="psT", bufs=2, space="PSUM") as psT,
            tc.tile_pool(name="psO", bufs=1, space="PSUM") as psO,
            tc.tile_pool(name="dram", bufs=1, space="DRAM") as dpool,
        ):
            # ---- constants ----
            wproj_sb = cpool.tile([128, 2, NPROJ], fp32)
            nc.sync.dma_start(
                wproj_sb[:], wproj[:, :].rearrange("(cc p) n -> p cc n", cc=2))
            bias_sb = cpool.tile([1, NPROJ], fp32)
            nc.sync.dma_start(bias_sb[:], bproj[:, :])
            wot_sb = cpool.tile([128, 2, C], bf16)
            nc.sync.dma_start(
                wot_sb[:], wot[:, :].rearrange("(kc p) n -> p kc n", kc=2))
            bnsc_sb = cpool.tile([128, 2], fp32)
            nc.sync.dma_start(bnsc_sb[:], bnsc[:, :])
            bnbi_sb = cpool.tile([128, 2], fp32)
            nc.sync.dma_start(bnbi_sb[:], bnbi[:, :])
            cent_sb = cpool.tile([128, 96], fp32)
            nc.sync.dma_start(cent_sb[:], cent[:, :])
            fold_sb = cpool.tile([128, 24], fp32)
            nc.sync.dma_start(fold_sb[:], fold[:, :])
            bias_rep = cpool.tile([128, NPROJ], fp32)
            nc.gpsimd.partition_broadcast(bias_rep[:], bias_sb[:])
            ident = cpool.tile([128, 128], fp32)
            make_identity(nc, ident[:])
            shmats = {}
            for sh in (1, W, W + 1):
                sa = cpool.tile([128, 128], bf16, tag=f"sha{sh}", name=f"sha{sh}")
                nc.gpsimd.memset(sa[:], 0.0)
                nc.gpsimd.affine_select(
                    out=sa[:], in_=sa[:], compare_op=A.not_equal, fill=1.0,
                    base=-sh, pattern=[[-1, 128]], channel_multiplier=1)
                sb_ = cpool.tile([128, 128], bf16, tag=f"shb{sh}", name=f"shb{sh}")
                nc.gpsimd.memset(sb_[:], 0.0)
                nc.gpsimd.affine_select(
                    out=sb_[:], in_=sb_[:], compare_op=A.not_equal, fill=1.0,
                    base=128 - sh, pattern=[[-1, 128]], channel_multiplier=1)
                shmats[sh] = (sa, sb_)

            # ---- persistent ----
            P_sb = ppool.tile([128, 49, 64], bf16)    # value, l=t*128+p rows
            nc.vector.memset(P_sb[:], 0.0)
            proj_sb = ppool.tile([128, NT, PCOLS], fp32, tag="bigb", name="proj_sb",
                                 padded_shape=[128, NT, PCOLS])
            C4 = [ppool.tile([128, NT * 64], bf16, tag=f"c4_{h}", name=f"c4_{h}") for h in (0, 1)]
            Rf = [ppool.tile([128, NT * 16], fp32, tag=f"rf_{h}", name=f"rf_{h}") for h in (0, 1)]
            IX = [ppool.tile([128, NT * 128], i16, tag=f"ix_{h}", name=f"ix_{h}") for h in (0, 1)]
            HO = ppool.tile([64, NT, 128], bf16)
            T_sb = ppool.tile([128, 2, NT, 128], bf16)
            T_dram = [dpool.tile([L, 128], bf16, tag=f"tab_{h}", name=f"tab_{h}") for h in (0, 1)]
            ho_b = [dpool.tile([512, 256], bf16, tag=f"hob{k}", name=f"hob{k}")
                    for k in (0, 1, 2)]
            a2a_o = [dpool.tile([512, 256], bf16, tag=f"a2o{k}", name=f"a2o{k}")
                     for k in (0, 1, 2)]

            def shift_tile(t):
                for blk, sh in enumerate((1, W, W + 1)):
                    sa, sb_ = shmats[sh]
                    psh = psT.tile([128, 64], fp32, tag="psh", name="psh")
                    nc.tensor.matmul(psh[:], sa[:], P_sb[:, t, :],
                                     start=True, stop=False)
                    nc.tensor.matmul(psh[:], sb_[:], P_sb[:, t + 1, :],
                                     start=False, stop=True)
                    nc.scalar.activation(
                        T_sb[:, :, t, (blk + 1) * 32:(blk + 2) * 32],
                        psh[:].rearrange("q (h e) -> q h e", h=2), AF.Copy)

            # ---- phase B: fused projections ----
            FTC = 4                      # tiles per feat chunk
            for t in range(NT):
                if t % FTC == 0:
                    ftc = wpool.tile([128, 2, FTC * 128], fp32, tag="ftc",
                                     bufs=2, name="ftc")
                    for cc in range(2):
                        nc.sync.dma_start(
                            ftc[:, cc, :],
                            feat[cc * 128:(cc + 1) * 128,
                                 t * 128:(t + FTC) * 128])
                ps = pspool.tile([128, NPROJ], fp32)
                for cc in range(2):
                    nc.tensor.matmul(
                        ps[:],
                        ftc[:, cc, (t % FTC) * 128:(t % FTC + 1) * 128],
                        wproj_sb[:, cc, :],
                        start=(cc == 0), stop=(cc == 1))
                nc.vector.tensor_tensor(out=ps[:], in0=ps[:],
                                        in1=bias_rep[:], op=A.add)
                nc.scalar.activation(P_sb[:, t, :], ps[:, 0:64], AF.Copy)
                nc.scalar.activation(
                    T_sb[:, :, t, 0:32],
                    ps[:, 0:64].rearrange("q (h e) -> q h e", h=2), AF.Copy)
                nc.scalar.activation(proj_sb[:, t, :], ps[:, 64:NPROJ], AF.Copy)
                if t >= 2:
                    shift_tile(t - 2)
            shift_tile(NT - 2)
            shift_tile(NT - 1)

            # ---- phase C/D helpers ----
            HNT = NT // 2

            def nonlin_half(t0):
                sl = proj_sb[:, t0:t0 + HNT, :]
                nc.scalar.activation(sl[:, :, 0:68], sl[:, :, 0:68],
                                     AF.Sigmoid)
                nc.vector.tensor_scalar(out=sl[:, :, 0:4], in0=sl[:, :, 0:4],
                                        scalar1=0.25, scalar2=0.75,
                                        op0=A.max, op1=A.min)
                nc.scalar.activation(sl[:, :, 68:100], sl[:, :, 68:100],
                                     AF.Exp)

            def weights_half(h, t0):
                shp = [128, HNT, 16]
                psl = proj_sb[:, t0:t0 + HNT, :]
                sx = psl[:, :, 2 * h:2 * h + 1]
                sy = psl[:, :, 2 * h + 1:2 * h + 2]
                anc = psl[:, :, 4 + 32 * h:4 + 32 * h + 32].rearrange(
                    "q t (p j) -> q t p j", j=2)
                ox, oy = anc[:, :, :, 0], anc[:, :, :, 1]
                att = psl[:, :, 68 + 16 * h:68 + 16 * h + 16]
                cx = cent_sb[:, t0:t0 + HNT]
                cy = cent_sb[:, 48 + t0:48 + t0 + HNT]

                axc = tpool.tile([128, HNT], fp32, tag="axc", name="axc")
                nc.vector.scalar_tensor_tensor(
                    out=axc[:], in0=sx[:, :, 0], scalar=-0.5, in1=cx,
                    op0=A.mult, op1=A.add)
                ayc = tpool.tile([128, HNT], fp32, tag="ayc", name="ayc")
                nc.vector.scalar_tensor_tensor(
                    out=ayc[:], in0=sy[:, :, 0], scalar=-0.5, in1=cy,
                    op0=A.mult, op1=A.add)

                def floorpath(o_ap, s_ap, a_t, scale, tagp):
                    tp = tpool.tile(shp, fp32, tag=f"tp{tagp}", name=f"tp{tagp}")
                    tr = tpool.tile(shp, fp32, tag=f"tr{tagp}", name=f"tr{tagp}")
                    tg = tpool.tile(shp, fp32, tag="tg", name=f"tg{tagp}")
                    nc.vector.tensor_tensor(
                        out=tp[:], in0=o_ap, in1=s_ap.to_broadcast(shp),
                        op=A.mult)
                    nc.vector.tensor_tensor(
                        out=tp[:], in0=tp[:],
                        in1=a_t[:][:, :, None].to_broadcast(shp), op=A.add)
                    nc.vector.tensor_scalar(out=tp[:], in0=tp[:],
                                            scalar1=0.0, scalar2=1.0,
                                            op0=A.max, op1=A.min)
                    nc.vector.tensor_scalar(out=tr[:], in0=tp[:],
                                            scalar1=scale, scalar2=TWO23,
                                            op0=A.mult, op1=A.add)
                    nc.vector.tensor_scalar(out=tr[:], in0=tr[:],
                                            scalar1=TWO23, scalar2=None,
                                            op0=A.subtract)
                    nc.vector.tensor_scalar(out=tp[:], in0=tp[:],
                                            scalar1=scale, scalar2=None,
                                            op0=A.mult)
                    nc.vector.tensor_tensor(out=tg[:], in0=tr[:], in1=tp[:],
                                            op=A.is_gt)
                    nc.vector.tensor_tensor(out=tr[:], in0=tr[:], in1=tg[:],
                                            op=A.subtract)     # floor
                    nc.vector.tensor_tensor(out=tp[:], in0=tp[:], in1=tr[:],
                                            op=A.subtract)     # frac
                    return tp, tr

                wx, x0 = floorpath(ox, sx, axc, float(W - 1), "x")
                wy, y0 = floorpath(oy, sy, ayc, float(H - 1), "y")

                rf = tpool.tile(shp, fp32, tag="tg", name="rf")
                nc.vector.scalar_tensor_tensor(
                    out=rf[:], in0=y0[:], scalar=float(W), in1=x0[:],
                    op0=A.mult, op1=A.add)
                # p-major table row: r' = (r % 128) * NT + r // 128
                qq = tpool.tile(shp, fp32, tag="qq", name="qq")
                gg = tpool.tile(shp, fp32, tag="gg2", name="gg2")
                nc.vector.tensor_scalar(out=qq[:], in0=rf[:],
                                        scalar1=1.0 / 128.0, scalar2=TWO23,
                                        op0=A.mult, op1=A.add)
                nc.vector.tensor_scalar(out=qq[:], in0=qq[:], scalar1=TWO23,
                                        scalar2=None, op0=A.subtract)
                nc.vector.tensor_scalar(out=gg[:], in0=rf[:],
                                        scalar1=1.0 / 128.0, scalar2=None,
                                        op0=A.mult)
                nc.vector.tensor_tensor(out=gg[:], in0=qq[:], in1=gg[:],
                                        op=A.is_gt)
                nc.vector.tensor_tensor(out=qq[:], in0=qq[:], in1=gg[:],
                                        op=A.subtract)          # r // 128
                nc.vector.scalar_tensor_tensor(
                    out=rf[:], in0=qq[:], scalar=-128.0, in1=rf[:],
                    op0=A.mult, op1=A.add)                      # r % 128
                rfh = Rf[h][:, t0 * 16:(t0 + HNT) * 16]
                nc.vector.scalar_tensor_tensor(
                    out=rfh.rearrange("q (t p) -> q t p", p=16),
                    in0=rf[:], scalar=float(NT), in1=qq[:],
                    op0=A.mult, op1=A.add)                      # r-prime

                ex = tpool.tile(shp, fp32, tag="ex", name="ex")
                nc.vector.tensor_scalar(out=ex[:], in0=wx[:], scalar1=-1.0,
                                        scalar2=1.0, op0=A.mult, op1=A.add)
                ey = tpool.tile(shp, fp32, tag="ey", name="ey")
                nc.vector.tensor_scalar(out=ey[:], in0=wy[:], scalar1=-1.0,
                                        scalar2=1.0, op0=A.mult, op1=A.add)

                asum = tpool.tile([128, HNT], fp32, tag="asum", name="asum")
                nc.vector.tensor_reduce(out=asum[:], in_=att,
                                        axis=mybir.AxisListType.X, op=A.add)
                arec = tpool.tile([128, HNT], fp32, tag="arec", name="arec")
                nc.vector.reciprocal(arec[:], asum[:])
                an = tpool.tile(shp, fp32, tag="an", name="an")
                nc.vector.tensor_tensor(
                    out=an[:], in0=att,
                    in1=arec[:][:, :, None].to_broadcast(shp), op=A.mult)
                m0 = tpool.tile(shp, fp32, tag="m0", name="m0")
                nc.vector.tensor_tensor(out=m0[:], in0=an[:], in1=ey[:],
                                        op=A.mult)
                nc.vector.tensor_tensor(out=an[:], in0=an[:], in1=wy[:],
                                        op=A.mult)             # an = m1
                c4v = C4[h][:, t0 * 64:(t0 + HNT) * 64].rearrange(
                    "q (t p s) -> q t p s", p=16, s=4)
                nc.vector.tensor_tensor(out=c4v[:, :, :, 0], in0=m0[:],
                                        in1=ex[:], op=A.mult)
                nc.vector.tensor_tensor(out=c4v[:, :, :, 1], in0=m0[:],
                                        in1=wx[:], op=A.mult)
                nc.vector.tensor_tensor(out=c4v[:, :, :, 2], in0=an[:],
                                        in1=ex[:], op=A.mult)
                nc.vector.tensor_tensor(out=c4v[:, :, :, 3], in0=an[:],
                                        in1=wx[:], op=A.mult)

                # fold R[q, (t,p)] -> IX[q%16, t*128 + p*8 + q//16] via PE
                for t in range(t0, t0 + HNT):
                    r8 = tpool.tile([128, 16, 8], fp32, tag="r8", name="r8")
                    nc.vector.tensor_tensor(
                        out=r8[:],
                        in0=Rf[h][:, t * 16:(t + 1) * 16][:, :, None]
                            .to_broadcast([128, 16, 8]),
                        in1=fold_sb[:, 16:24][:, None, :]
                            .to_broadcast([128, 16, 8]),
                        op=A.mult)
                    psI = psT.tile([32, 128], fp32, tag="pstX", name="psI")
                    nc.tensor.matmul(psI[0:16, :], fold_sb[:, 0:16],
                                     r8[:].rearrange("k p e -> k (p e)"),
                                     start=True, stop=True)
                    nc.scalar.activation(
                        IX[h][0:16, t * 128:(t + 1) * 128], psI[0:16, :],
                        AF.Copy)
                csl = slice(t0 * 128, (t0 + HNT) * 128)
                nc.sync.dma_start(IX[h][16:32, csl], IX[h][0:16, csl])
                nc.sync.dma_start(IX[h][32:64, csl], IX[h][0:32, csl])
                nc.sync.dma_start(IX[h][64:128, csl], IX[h][0:64, csl])

            # ---- phase C: per-half nonlinearities + weights + indices ----
            nonlin_half(0)
            weights_half(0, 0)
            nonlin_half(HNT)
            weights_half(0, HNT)
            for h in (0, 1):
                nc.sync.dma_start(
                    T_dram[h][:, :].rearrange("(p t) e -> p (t e)", p=128),
                    T_sb[:, h, :, :])

            # ---- phase E: gather + combine + point-reduce (per head) ----
            rhs_sb = ppool.tile([128, 4, LQ], bf16)
            out_sb = ppool.tile([128, 4, LQ], fp32, tag="bigb", name="out_sb",
                                padded_shape=[128, 4, NT * PCOLS // 4])

            def chunk_comm(k):
                # core j's columns are tiles {8i+j}; chunk k covers i in
                # [2k, 2k+2) -> tiles [16k, 16k+16)
                for j in range(8):
                    nc.scalar.dma_start(
                        ho_b[k][j * 64:(j + 1) * 64, :].rearrange(
                            "r (i e) -> r i e", e=128),
                        HO[:, 16 * k + j:16 * (k + 1):8, :])
                nc.gpsimd.collective_compute(
                    "AllToAll",
                    A.bypass,
                    replica_groups=[[0, 1, 2, 3, 4, 5, 6, 7]],
                    ins=[ho_b[k].opt()],
                    outs=[a2a_o[k].opt()],
                )
                nc.sync.dma_start(
                    rhs_sb[:, :, k * 256:(k + 1) * 256],
                    a2a_o[k][:, :].rearrange(
                        "(bb kc p) n -> p (bb kc) n", bb=2, kc=2))

            def chunk_mm(k):
                for bb in range(2):
                    for cc in range(2):
                        pso = psO.tile([128, 256], fp32, name="pso")
                        for kc in range(2):
                            nc.tensor.matmul(
                                pso[:],
                                wot_sb[:, kc, cc * 128:(cc + 1) * 128],
                                rhs_sb[:, 2 * bb + kc,
                                       k * 256:(k + 1) * 256],
                                start=(kc == 0), stop=(kc == 1))
                        nc.vector.tensor_scalar(
                            out=out_sb[:, 2 * bb + cc, k * 256:(k + 1) * 256],
                            in0=pso[:],
                            scalar1=bnsc_sb[:, cc:cc + 1],
                            scalar2=bnbi_sb[:, cc:cc + 1],
                            op0=A.mult, op1=A.add)
                nc.sync.dma_start(
                    out[:, :].rearrange("(q p) n -> p q n", q=4)[
                        :, :, k * 256:(k + 1) * 256],
                    out_sb[:, :, k * 256:(k + 1) * 256])

            qctr = 0
            for h in (0, 1):
                for t in range(NT):
                    G = wpool.tile([128, 16, 128], bf16, tag="G", bufs=10)
                    nc.gpsimd.dma_gather(
                        out_ap=G[:],
                        in_ap=T_dram[h][:, :],
                        idxs_ap=IX[h][:, t * 128:(t + 1) * 128],
                        num_idxs=2048,
                        num_idxs_reg=2048,
                        elem_size=128,
                        single_packet=False,
                        queue_num=qctr % 4,
                    )
                    qctr += 1
                    cw = wpool.tile([128, 16, 4, 32], bf16, tag="cw",
                                    bufs=2, name="cw")
                    nc.scalar.activation(
                        cw[:],
                        C4[h][:, t * 64:(t + 1) * 64].rearrange(
                            "q (b s) -> q b s", s=4)[:, :, :, None]
                            .to_broadcast([128, 16, 4, 32]),
                        AF.Copy)
                    nc.vector.tensor_tensor(
                        out=G[:], in0=G[:],
                        in1=cw[:].rearrange("q b s d -> q b (s d)"),
                        op=A.mult)
                    gf = G[:].rearrange("q b e -> q (b e)")
                    F1 = wpool.tile([128, 1024], bf16, tag="F1", bufs=2)
                    nc.vector.tensor_tensor(out=F1[:], in0=gf[:, 0:1024],
                                            in1=gf[:, 1024:2048], op=A.add)
                    F2 = wpool.tile([128, 512], bf16, tag="F2", bufs=3)
                    nc.vector.tensor_tensor(out=F2[:], in0=F1[:, 0:512],
                                            in1=F1[:, 512:1024], op=A.add)
                    F3 = wpool.tile([128, 256], bf16, tag="F3", bufs=3)
                    nc.vector.tensor_tensor(out=F3[:], in0=F2[:, 0:256],
                                            in1=F2[:, 256:512], op=A.add)
                    F4 = wpool.tile([128, 128], bf16, tag="F4", bufs=3)
                    nc.vector.tensor_tensor(out=F4[:], in0=F3[:, 0:128],
                                            in1=F3[:, 128:256], op=A.add)
                    F5 = wpool.tile([128, 64], bf16, tag="F5", bufs=3)
                    nc.vector.tensor_tensor(out=F5[:], in0=F4[:, 0:64],
                                            in1=F4[:, 64:128], op=A.add)
                    hot = tpool.tile([128, 32], fp32, tag="hot", name="hot",
                                     bufs=3)
                    nc.vector.tensor_tensor(out=hot[:], in0=F5[:, 0:32],
                                            in1=F5[:, 32:64], op=A.add)
                    pst = psT.tile([32, 128], fp32, tag="pstX", name="pst")
                    nc.tensor.matmul(pst[:], hot[:], ident[:],
                                     start=True, stop=True)
                    nc.scalar.activation(HO[h * 32:(h + 1) * 32, t, :],
                                         pst[:], AF.Copy)
                    if h == 0 and t == 8:
                        weights_half(1, 0)
                    if h == 0 and t == 20:
                        weights_half(1, HNT)
                    if h == 1 and t == 15:
                        chunk_comm(0)
                    if h == 1 and t == 31:
                        chunk_comm(1)
            chunk_comm(2)
            chunk_mm(0)
            chunk_mm(1)
            chunk_mm(2)

    nc.finalize()
    return nc


def _prep_inputs(inputs):
    f = np.float32
    feat_sd = np.asarray(inputs['feat_sd'], dtype=f)
    w_size = np.asarray(inputs['w_size'], dtype=f)
    b_size = np.asarray(inputs['b_size'], dtype=f)
    w_anchor = np.asarray(inputs['w_anchor'], dtype=f)
    b_anchor = np.asarray(inputs['b_anchor'], dtype=f)
    w_value = np.asarray(inputs['w_value'], dtype=f)
    b_value = np.asarray(inputs['b_value'], dtype=f)
    w_att = np.asarray(inputs['w_att'], dtype=f)
    b_att = np.asarray(inputs['b_att'], dtype=f)
    w_out = np.asarray(inputs['w_out'], dtype=f)
    bn_gamma = np.asarray(inputs['bn_gamma'], dtype=f)
    bn_beta = np.asarray(inputs['bn_beta'], dtype=f)
    bn_mean = np.asarray(inputs['bn_mean'], dtype=f)
    bn_var = np.asarray(inputs['bn_var'], dtype=f)

    import ml_dtypes
    wot = np.ascontiguousarray(w_out.T).astype(ml_dtypes.bfloat16)
    scale = (bn_gamma / np.sqrt(bn_var + np.float32(1e-5))).astype(f)
    bias = (bn_beta - bn_mean * scale).astype(f)
    bnsc = np.ascontiguousarray(scale.reshape(2, 128).T)
    bnbi = np.ascontiguousarray(bias.reshape(2, 128).T)

    k = np.arange(128)
    foldm = np.zeros((128, 24), np.float32)
    foldm[k, k % 16] = 1.0
    foldm[k, 16 + k // 16] = 1.0

    l = np.arange(L).reshape(NT, 128)
    cx = ((l % W + 0.5).astype(f) / np.float32(W + EPS)).T
    cy = ((l // W + 0.5).astype(f) / np.float32(H + EPS)).T
    cent = np.ascontiguousarray(np.concatenate([cx, cy], axis=1), dtype=f)

    in_maps = []
    for m in range(8):
        b = m // 4
        h0 = 2 * (m % 4)
        h1 = h0 + 1
        wrows = np.concatenate([
            w_value[h0 * 32:(h0 + 1) * 32],
            w_value[h1 * 32:(h1 + 1) * 32],
            w_size[[2 * h0, 2 * h0 + 1, 2 * h1, 2 * h1 + 1]],
            w_anchor[h0 * 32:(h0 + 1) * 32],
            w_anchor[h1 * 32:(h1 + 1) * 32],
            w_att[h0 * 16:(h0 + 1) * 16],
            w_att[h1 * 16:(h1 + 1) * 16],
        ], axis=0)
        brows = np.concatenate([
            b_value[h0 * 32:(h0 + 1) * 32],
            b_value[h1 * 32:(h1 + 1) * 32],
            b_size[[2 * h0, 2 * h0 + 1, 2 * h1, 2 * h1 + 1]],
            b_anchor[h0 * 32:(h0 + 1) * 32],
            b_anchor[h1 * 32:(h1 + 1) * 32],
            b_att[h0 * 16:(h0 + 1) * 16],
            b_att[h1 * 16:(h1 + 1) * 16],
        ], axis=0)
        in_maps.append({
            "feat": np.ascontiguousarray(feat_sd[b].reshape(C, L)),
            "wproj": np.ascontiguousarray(wrows.T),
            "bproj": np.ascontiguousarray(brows.reshape(1, NPROJ)),
            "wot": wot,
            "bnsc": bnsc,
            "bnbi": bnbi,
            "cent": cent,
            "fold": foldm,
        })
    return in_maps


def _run(inputs, trace=False):
    from concourse.bass_utils import run_bass_kernel_spmd
    if "nc" not in _CACHE:
        _CACHE["nc"] = _build_nc()
    nc = _CACHE["nc"]
    in_maps = _prep_inputs(inputs)
    res = run_bass_kernel_spmd(nc, in_maps, core_ids=list(range(8)),
                               trace=trace)
    full = np.empty((B, C, L), np.float32)
    for m in range(8):
        o = res.results[m]["out"].reshape(2, C, 6, 128)
        for bb in range(2):
            for i in range(6):
                t = 8 * i + m
                full[bb][:, t * 128:(t + 1) * 128] = o[bb, :, i]
    return full.reshape(B, C, H, W), res.exec_time_ns


def kernel(**inputs):
    out, _ = _run(inputs, trace=False)
    return out
